# revision 21
# baseline (speedup 1.0000x reference)
"""Trainium2 Bass kernel for nn_AttnConv2d (sparse_attention).

Reference math (per sample b):
  y1 = conv3x3(x, W1), y2 = conv3x3(x, W2), y3 = conv3x3(x, W3)   (same pad)
  For each of the 9 offsets k=(kh,kw): the "grid_k" pixels are
  (3*hb+kh, 3*wb+kw).  z[d, k, c] = sum_{l in grid_k} y1[c, l] * y2[d, l].
  P = softmax over (k,c) of z / sqrt(864), per row d.
  out[d, h, w] = sum_{c, (i,j)} P[d, i*3+j, c] * y3[c, h+i-1, w+j-1]
  (i.e. a 3x3 conv of y3 with per-sample weights from the softmax).

Implementation notes:
 - Data parallel over batch: 16 samples -> 8 cores x 2 samples.
 - Matmul dtype is configurable: float16 (default; 1 PE cycle/row,
   ~1e-3 rel err) or float32r (~4e-4 rel err but the embedded weight
   load of fp32-class matmuls serializes, costing ~30%).
 - fp32r operands must be produced by a rounding compute op (DVE copy),
   never directly by DMA; fp16 has no such constraint so inputs DMA in
   pre-converted from the host.
 - conv1/conv2 are computed in grid-k pixel order; their [l, c]
   transposed chunks (gram operands need pixels on partitions) are made
   by the DMA XBAR (dma_start_transpose, fp16 SBUF->SBUF, one batched
   trigger per conv tile) so they cost no PE cycles.  A deep (14) st
   staging ring absorbs the slow-transpose window while the x-prefetch
   shares the DMA engines.
 - PE program order per sample: conv1+conv2 -> gram -> conv3
   -> E transposes -> apply.  conv3 sits between gram and E-transposes
   so the softmax (ACT/DVE) latency hides under PE work.
 - ~5us of dummy warmup matmuls at kernel start keep the PE busy through
   the input-DMA lead-in so the HAM clock gate is at 2.4 GHz (not the
   cold 1.2) when the first real conv matmul issues.
 - Sample-0 band A is split across both hwdge queues (scalar + sync
   trigger engines) so it lands ~1.2us sooner; the first real matmul
   fires ~14us in, bounded by NEFF preamble (~7us) + DMA ring startup
   (~2us) + the 1.2MB critical transfer at ~250 GB/s.
 - Stationary weights are host-padded to 128 couts (zeros beyond 96) to
   hit the compiler's fast-weight-load path; psum tiles are [128, n] with
   consumers reading [:96].
 - PSUM banks: conv 2 + conv3/apply 3 + E-transpose 1 + gram 2 (1 buf
   serialized the gram on its z copy).
 - Measured: PE issue stream is gapless (<2us total idle over 312us);
   conv matmuls run at ~228ns/512cols (~6% over the 213.5 streaming
   floor, from per-row-jump cost of the strided x-window reads — padding
   the stream to whole rows costs more than the jumps save).
"""

import contextlib
import math

import numpy as np

import concourse.bacc as bacc
import concourse.bass as bass
import concourse.mybir as mybir
import concourse.tile as tile

F32 = mybir.dt.float32
F32R = mybir.dt.float32r
FP16 = mybir.dt.float16

B, C, H, W = 16, 96, 96, 96
NCORES = 8
S = B // NCORES  # samples per core
MM_DT = FP16  # conv / apply matmuls (fp16 ~ fp32r speed, no rounding funnel)
TR_DT = FP16  # conv1/2-output transposes + gram matmuls


def build_program(
    s_per_core=S, h=H, w=W, mm_dt=MM_DT, reps=1, hw_loop=None, ablate=""
):
    hp, wp = h + 2, w + 2
    hb, wb = h // 3, w // 3  # grid blocks per offset k
    gp, gw = hb + 2, wb + 2  # zero-padded grid block planes
    padpix = 9 * gp * gw  # x arrives in grid layout [9, gp, gw]
    L = hb * wb  # pixels per grid
    nch = (L + 127) // 128  # 128-pixel transpose chunks per grid
    # conv tiling in grid order: gh block-rows per PSUM tile (<=512 px)
    gh = min(hb, 512 // wb)
    n_gt = (hb + gh - 1) // gh
    gn = gh * wb
    assert gn % 128 == 0 or n_gt == 1, (gn, n_gt)
    # apply natural-order tiling: rt rows per tile (<=512 px), ragged tail
    rt = max(1, min(h, 512 // w))
    tiles_rt = [(h0, min(rt, h - h0)) for h0 in range(0, h, rt)]
    n_rt = len(tiles_rt)
    sc = 1.0 / math.sqrt(C * 9)
    rounded = mm_dt == F32R  # fp32r needs DVE-rounded producers
    in_dt = F32 if rounded else mm_dt

    split_x = "nox" not in ablate and not rounded and hb // gh == 2
    nc = bacc.Bacc("TRN2", debug=False, enable_asserts=False)
    if split_x:
        # x arrives as two contiguous row bands (A: rows [0,gh+2) for g=0,
        # B: rows [gh,gp) for g=1) so the input DMAs use big descriptors.
        xpa_d = nc.dram_tensor(
            "xpa", [s_per_core, C, 9, gh + 2, gw], in_dt, kind="ExternalInput"
        )
        xpb_d = nc.dram_tensor(
            "xpb", [s_per_core, C, 9, gp - gh, gw], in_dt, kind="ExternalInput"
        )
        xp_d = None
    else:
        xp_d = nc.dram_tensor(
            "xp", [s_per_core, C, 9, gp, gw], in_dt, kind="ExternalInput"
        )
    # weights host-padded to 128 couts (zeros beyond C): a 128-column
    # stationary triggers the compiler's Fast Weight Load path, halving
    # LDWEIGHTS time so it fully hides under the matmul stream.
    MP = 128
    w_d = [
        nc.dram_tensor(f"w{i}t", [C, 9, MP], in_dt, kind="ExternalInput")
        for i in (1, 2, 3)
    ]
    id_d = nc.dram_tensor("ident", [C, C], in_dt, kind="ExternalInput")
    out_d = nc.dram_tensor("out", [s_per_core, C, h, w], F32, kind="ExternalOutput")

    xp_f = None if split_x else xp_d.ap().rearrange("s c a h w -> s c (a h w)")
    out_f = out_d.ap().rearrange("s c h w -> s c (h w)")

    xchunk = (padpix + 3) // 4
    cfg = dict(
        h=h, w=w, hp=hp, wp=wp, gp=gp, gw=gw, padpix=padpix, xchunk=xchunk,
        hb=hb, wb=wb, L=L, nch=nch, gh=gh, n_gt=n_gt, gn=gn, rt=rt, n_rt=n_rt,
        tiles_rt=tiles_rt,
        sc=sc, mm_dt=mm_dt, rounded=rounded, ablate=ablate, split_x=split_x,
    )

    with tile.TileContext(nc) as tc:
        with (
            tc.tile_pool(name="consts", bufs=1) as consts,
            tc.tile_pool(name="stage", bufs=2) as stage,
            tc.tile_pool(name="big", bufs=1) as big,
            tc.tile_pool(name="work", bufs=3) as work,
            tc.tile_pool(name="sm", bufs=1) as smp,
            tc.tile_pool(name="psmm", bufs=2, space="PSUM") as psmm,
            tc.tile_pool(name="psma", bufs=3, space="PSUM") as psma,
            tc.tile_pool(name="pstr", bufs=1, space="PSUM") as pstr,
            tc.tile_pool(name="psgr", bufs=2, space="PSUM") as psgr,
        ):
            # E transposes share the pstr ring (same tag) — frees a PSUM
            # bank so the gram pool gets 2 bufs (its copy was serializing).
            pools = dict(
                consts=consts, stage=stage, big=big, work=work, smp=smp,
                psmm=psmm, psma=psma, pstr=pstr, pstre=pstr, psgr=psgr,
            )
            if "nowarm" not in ablate:
                # ~5us of dummy matmuls at kernel start: keeps PE busy
                # through the input-DMA lead-in so the HAM clock gate is
                # warm (2.4 GHz) before the first real conv matmul.
                warm = consts.tile([128, 96], mm_dt, tag="warm", name="warm")
                nc.vector.memset(warm, 0.0)
                wps = psgr.tile([96, 96], F32, tag="gram", name="wps")
                for _ in range(120):
                    nc.tensor.matmul(wps, warm[:, :96], warm, start=True, stop=True)
            # ---- constants: weights (host-transposed [cin, d, cout]), identity
            x0 = None
            w_sb = []
            for i, wd in enumerate(w_d):
                if rounded:
                    ws = stage.tile([C, 9 * MP], F32, tag="wstage", name=f"ws{i}")
                    nc.sync.dma_start(
                        out=ws, in_=wd.ap().rearrange("c a d -> c (a d)")
                    )
                    wt = consts.tile([C, 9, MP], mm_dt, tag=f"w{i}", name=f"w{i}")
                    nc.vector.tensor_copy(wt.rearrange("c a d -> c (a d)"), ws)
                else:
                    wt = consts.tile([C, 9, MP], mm_dt, tag=f"w{i}", name=f"w{i}")
                    nc.sync.dma_start(out=wt, in_=wd.ap())
                w_sb.append(wt)
                if i == 0 and split_x:
                    # sample-0 band A split across BOTH hwdge queues (each
                    # trigger engine owns one queue): scalar's queue takes 5
                    # planes while sync's takes w1 then 4 planes — band A
                    # lands ~2.5us sooner than single-queue serial.
                    xa0 = big.tile(
                        [C, 9, gh + 2, gw], mm_dt, tag="x_pad_a", bufs=2,
                        name="xa",
                    )
                    xb0 = big.tile(
                        [C, 9, gp - gh, gw], mm_dt, tag="x_pad_b", bufs=2,
                        name="xb",
                    )
                    nc.scalar.dma_start(out=xa0[:, 0:5], in_=xpa_d.ap()[0][:, 0:5])
                    nc.sync.dma_start(out=xa0[:, 5:9], in_=xpa_d.ap()[0][:, 5:9])
                    x0 = (xa0, xb0)
                if i == 1 and split_x:
                    # xb after w2 on sync: deadline is the first g=1 tile
                    # (~35us in); w3/ident can trail it.
                    nc.sync.dma_start(out=x0[1], in_=xpb_d.ap()[0])
            ids = stage.tile([C, C], F32, tag="wstage", name="ids")
            if rounded:
                nc.sync.dma_start(out=ids, in_=id_d.ap())
                ident = consts.tile([C, C], mm_dt, tag="ident")
                nc.vector.tensor_copy(ident, ids)
            else:
                ident = consts.tile([C, C], mm_dt, tag="ident")
                # scalar queue: lands before sync finishes w2/w3, so the
                # first conv transposes (~4us after first matmul) aren't
                # gated on the weight-DMA tail.
                nc.scalar.dma_start(out=ident, in_=id_d.ap())
                nc.vector.tensor_copy(ids, ident)
            ident_h = consts.tile([C, C], TR_DT, tag="ident_h")
            nc.vector.tensor_copy(ident_h, ids)
            zerof = consts.tile([C, max(hp, wp)], F32, tag="zerof")
            nc.vector.memset(zerof, 0.0)

            # ---- persistent per-core buffers
            y3_pad = big.tile([C, hp, wp], mm_dt, tag="y3_pad")
            # apply stationary [C, 9, 128]: cols >= C stay zero so the
            # 128-col weight load (FWL) is valid; ACT writes [:, k, :C].
            et_b = big.tile([C, 9, 128], mm_dt, tag="et_b")
            nc.vector.memset(et_b.rearrange("c a d -> c (a d)"), 0.0)
            y1t = big.tile([128, 9, nch, C], TR_DT, tag="y1t")
            y2t = big.tile([128, 9, nch, C], TR_DT, tag="y2t")
            bufs = dict(
                y3_pad=y3_pad, y1t=y1t, y2t=y2t,
                y1t_f=y1t.rearrange("p a b c -> p (a b c)"), et_b=et_b,
                w_sb=w_sb, ident=ident, ident_h=ident_h,
                xp_f=xp_f, out_f=out_f, x0=x0,
                xpa=xpa_d.ap() if split_x else None,
                xpb=xpb_d.ap() if split_x else None,
            )

            # y3_pad borders are zero once (never overwritten: conv3 writes
            # the interior only, and the single buffer persists).
            nc.vector.tensor_copy(y3_pad[:, 0, :], zerof[:, :wp])
            nc.vector.tensor_copy(y3_pad[:, hp - 1, :], zerof[:, :wp])
            nc.vector.tensor_copy(y3_pad[:, :, 0], zerof[:, :hp])
            nc.vector.tensor_copy(y3_pad[:, :, wp - 1], zerof[:, :hp])

            if "nox" in ablate:
                # load x once, outside the timing loop
                x_pad1 = big.tile([C, 9, gp, gw], mm_dt, tag="x_pad", name="x_pad1")
                xf = x_pad1.rearrange("c a h w -> c (a h w)")
                for o in range(0, padpix, xchunk):
                    cl = min(xchunk, padpix - o)
                    xs = stage.tile([C, xchunk], F32, tag="xs", name="xs")
                    nc.sync.dma_start(out=xs[:, :cl], in_=xp_f[0][:, o : o + cl])
                    nc.vector.tensor_copy(xf[:, o : o + cl], xs[:, :cl])
                bufs["x_pad_static"] = x_pad1
            if "noc12" in ablate:
                et_s = big.tile([C, 9, C], mm_dt, tag="et_s", name="et_s")
                for k in range(9):
                    nc.vector.tensor_copy(et_s[:, k, :], zerof[:, :C])
                rinv_s = big.tile([C, 1], F32, tag="rinv_s", name="rinv_s")
                nc.vector.memset(rinv_s, 1.0)
                bufs["et_static"], bufs["rinv_static"] = et_s, rinv_s

            loop_ctx = (
                tc.For_i(0, hw_loop, 1)
                if hw_loop is not None
                else contextlib.nullcontext()
            )
            with loop_ctx:
                for s in [s for _ in range(reps) for s in range(s_per_core)]:
                    _emit_sample(nc, s, pools, bufs, cfg)
    nc.finalize()
    return nc


def _emit_sample(nc, s, pools, bufs, cfg):
    stage, work, smp, big = pools["stage"], pools["work"], pools["smp"], pools["big"]
    psmm, psma, pstr, pstre, psgr = (
        pools["psmm"], pools["psma"], pools["pstr"], pools["pstre"], pools["psgr"]
    )
    y3_pad, y1t, y2t = bufs["y3_pad"], bufs["y1t"], bufs["y2t"]
    y1t_f, w_sb, ident, ident_h = (
        bufs["y1t_f"], bufs["w_sb"], bufs["ident"], bufs["ident_h"]
    )
    xp_f, out_f = bufs["xp_f"], bufs["out_f"]
    hp, wp, gp, gw, padpix, xchunk = (
        cfg["hp"], cfg["wp"], cfg["gp"], cfg["gw"], cfg["padpix"], cfg["xchunk"]
    )
    w = cfg["w"]
    wb, L, nch, gh, n_gt, gn, rt, n_rt, sc = (
        cfg["wb"], cfg["L"], cfg["nch"], cfg["gh"], cfg["n_gt"], cfg["gn"],
        cfg["rt"], cfg["n_rt"], cfg["sc"],
    )
    mm_dt, rounded, ablate = cfg["mm_dt"], cfg["rounded"], cfg["ablate"]

    # ---- load x (host sends the zero-padded GRID layout [9, gp, gw]; fp32r
    # needs a DVE rounding hop).  Double-buffered so the next sample's load
    # overlaps compute.  The load is split into two row-band tiles (g=0
    # rows / g=1 rows, 1-row overlap) on separate DMAs, so with the
    # g-outer conv loop below PE starts after the first band lands.
    split_x = cfg["split_x"]
    if "nox" in ablate:
        x_pad = bufs["x_pad_static"]
    elif split_x and s == 0 and bufs.get("x0") is not None:
        xa, xb = bufs["x0"]  # preloaded at program start
    elif split_x:
        ra = gh + 2  # band A rows [0, gh+2): covers g=0 reads (q in -1..1)
        rb = gp - gh  # band B rows [gh, gp): covers g=1 reads
        xa = big.tile([C, 9, ra, gw], mm_dt, tag="x_pad_a", bufs=2, name="xa")
        xb = big.tile([C, 9, rb, gw], mm_dt, tag="x_pad_b", bufs=2, name="xb")
        # band A feeds the first PE work: 3 contiguous-plane DMAs on separate
        # queues so it lands ~3x sooner; band B has a late deadline (g=1).
        for a0 in range(0, 9, 3):
            nc.sync.dma_start(
                out=xa[:, a0 : a0 + 3], in_=bufs["xpa"][s][:, a0 : a0 + 3]
            )
        nc.sync.dma_start(out=xb, in_=bufs["xpb"][s])
    else:
        x_pad = big.tile([C, 9, gp, gw], mm_dt, tag="x_pad", bufs=2, name="x_pad")
        x_pad_f = x_pad.rearrange("c a h w -> c (a h w)")
        if rounded:
            for o in range(0, padpix, xchunk):
                cl = min(xchunk, padpix - o)
                xs = stage.tile([C, xchunk], F32, tag="xs", name="xs")
                nc.sync.dma_start(out=xs[:, :cl], in_=xp_f[s][:, o : o + cl])
                nc.vector.tensor_copy(x_pad_f[:, o : o + cl], xs[:, :cl])
        else:
            nc.sync.dma_start(out=x_pad_f, in_=xp_f[s])

    # Contiguous (pad-stripped) copies of the x bands: 63/81 tap-streams
    # have zero column shift (qc==0) and can stream [gh, wb] windows with
    # NO row jumps from these, dropping the ~13ns/MM strided-read tax on
    # 7/9 of the conv matmuls.  The qc=+-1 edge taps keep reading the
    # padded bands.  Built by two DVE copies (~2.2us each) right after
    # the band DMAs; sample 0's first conv tiles read the padded path
    # while the copy drains.
    use_xc = split_x and "noxc" not in ablate
    if use_xc:
        ra_, rb_ = gh + 2, gp - gh
        xa_c = big.tile([C, 9, ra_, wb], mm_dt, tag="xa_c", bufs=2, name="xa_c")
        nc.vector.tensor_copy(xa_c, xa[:, :, :, 1 : 1 + wb])
        xb_c = big.tile([C, 9, rb_, wb], mm_dt, tag="xb_c", bufs=2, name="xb_c")
        # ACT, not DVE: DVE is in-order and this waits on band B's DMA
        # (~20us) — on DVE it would block the conv CASTs behind it.
        nc.scalar.copy(xb_c, xb[:, :, :, 1 : 1 + wb])

    def grid_rhs(k, g, d, allow_c=True):
        """x view for output-grid k, block-row tile g, conv offset d."""
        kh, kw = divmod(k, 3)
        dh, dw = divmod(d, 3)
        q, r = divmod(kh + dh - 1, 3)
        qc, rc = divmod(kw + dw - 1, 3)
        h0 = g * cfg["gh"]
        if split_x:
            r0 = 1 + h0 + q - (0 if g == 0 else gh)
            if qc == 0 and allow_c and use_xc:
                srcc = xa_c if g == 0 else xb_c
                return srcc[:, r * 3 + rc, r0 : r0 + gh, :]
            src = xa if g == 0 else xb
            return src[:, r * 3 + rc, r0 : r0 + gh, 1 + qc : 1 + qc + wb]
        return x_pad[
            :, r * 3 + rc, 1 + h0 + q : 1 + h0 + q + gh, 1 + qc : 1 + qc + wb
        ]

    # ---- conv1 + conv2 in grid-k order, transposed into y1t/y2t.  The
    # transposes go through the DMA XBAR (dma_start_transpose, fp16
    # SBUF->SBUF) so they cost no PE cycles; triggers alternate between
    # the two hwdge queues (sync/scalar), which are otherwise idle in the
    # conv phase.  "nodmatr" ablate falls back to PE transposes + DVE
    # copies (one conv-tile behind so PE never waits on the CAST).
    n_ch_g = (gn + 127) // 128
    use_dmatr = (
        "nodmatr" not in ablate and gn % 128 == 0 and C % 16 == 0
    )

    def emit_transposes(pending):
        st, k_, yt_, g_ = pending
        ci0 = g_ * n_ch_g
        if use_dmatr:
            eng = nc.sync if k_ % 2 == 0 else nc.scalar
            if "dmatr1" in ablate:  # per-chunk 2D fallback
                for j in range(n_ch_g):
                    eng.dma_start_transpose(
                        out=yt_[:, k_, ci0 + j, :],
                        in_=st[:, j * 128 : (j + 1) * 128],
                    )
            else:  # batched: one trigger per tile, 3D out AP
                eng.dma_start_transpose(
                    out=yt_[:, k_, ci0 : ci0 + n_ch_g, :], in_=st
                )
            return
        pt = pstr.tile([128, n_ch_g, C], TR_DT, tag="tr", name="pt")
        for j, j0 in enumerate(range(0, gn, 128)):
            cs = min(128, gn - j0)
            nc.tensor.transpose(pt[:cs, j, :], st[:, j0 : j0 + cs], ident_h)
        cs0 = min(128, gn - (n_ch_g - 1) * 128)  # partitions of last chunk
        if cs0 == 128:
            nc.vector.tensor_copy(
                yt_[:, k_, ci0 : ci0 + n_ch_g, :].rearrange("p a c -> p (a c)"),
                pt.rearrange("p a c -> p (a c)"),
            )
        else:  # small-config fallback: per-chunk copies of valid partitions
            for j in range(n_ch_g):
                cs = min(128, gn - j * 128)
                nc.vector.tensor_copy(yt_[:cs, k_, ci0 + j, :], pt[:cs, j, :])

    # g-outer so all g=0 tiles run off band A while band B still streams in.
    # Sample 0's first tiles keep the padded-read path so the PE isn't
    # gated on the pad-strip DVE copy (~2us after band A lands).
    pending = None
    tile_no = 0
    for g in range(n_gt) if "noc12" not in ablate else ():
        for k in range(9):
            for wt, yt in ((w_sb[0], y1t), (w_sb[1], y2t)):
                allow_c = s > 0 or g == 1
                tile_no += 1
                pc = psmm.tile([128, gn], F32, tag="mm", name="pc")
                for d in range(9):
                    nc.tensor.matmul(
                        pc, wt[:, d, :], grid_rhs(k, g, d, allow_c),
                        start=(d == 0), stop=(d == 8),
                    )
                # 10-deep staging ring: during the first ~30us the XBAR
                # transpose transfers share DMA engines with the 4MB
                # x-prefetch and run slow; a deep ring keeps the CASTs (and
                # therefore the psum banks and conv matmuls) from blocking
                # on them.  Deadline is only the gram (~75us in).
                st = work.tile([C, gn], TR_DT, tag="st", name="st", bufs=14)
                nc.vector.tensor_copy(st, pc[:C])
                if pending is not None:
                    emit_transposes(pending)
                pending = (st, k, yt, g)
    if pending is not None:
        emit_transposes(pending)

    # ---- gram: z[d, k, c] = sum_l y2t[l,d] * y1t[l,c]
    z = smp.tile([C, 9, C], F32, tag="z", name="z")
    gram_n = C  # fp16 gram runs full-rate at any moving size
    for k in range(9) if "noc12" not in ablate else ():
        pg = psgr.tile([C, gram_n], F32, tag="gram", name="pg")
        for ci in range(nch):
            cs = min(128, L - ci * 128)
            off = (k * nch + ci) * C
            n = min(gram_n, 9 * nch * C - off)
            nc.tensor.matmul(
                pg[:, :n],
                y2t[:cs, k, ci, :],
                y1t_f[:cs, off : off + n],
                start=(ci == 0),
                stop=(ci == nch - 1),
            )
        nc.vector.tensor_copy(z[:, k, :], pg[:, :C])

    # ---- softmax over (k, c) per row d; normalization deferred to the
    # output scale (rinv folded into the final psum->sbuf copy).
    if "noc12" not in ablate:
        zf = z.rearrange("d a c -> d (a c)")
        mneg = smp.tile([C, 1], F32, tag="mneg", name="mneg")
        nc.vector.reduce_max(
            out=mneg, in_=zf, axis=mybir.AxisListType.X, negate=True
        )
        mns = smp.tile([C, 1], F32, tag="mns", name="mns")
        nc.vector.tensor_scalar_mul(mns, mneg, sc)
        e = smp.tile([C, 9, C], F32, tag="e", name="e")
        r = smp.tile([C, 1], F32, tag="r", name="r")
        nc.scalar.activation(
            e.rearrange("d a c -> d (a c)"),
            zf,
            mybir.ActivationFunctionType.Exp,
            bias=mns,
            scale=sc,
            accum_out=r,
        )
        rinv = smp.tile([C, 1], F32, tag="rinv", name="rinv")
        nc.vector.reciprocal(rinv, r)
        # e2: ACT-produced rounding copy; the whole conv3/apply side (y3, et,
        # e2, output scaling) funnels through ACT so each matmul there waits
        # on a single semaphore, while conv1/2+gram funnel through DVE.
        e2 = smp.tile([C, 9, C], mm_dt, tag="e2", name="e2")
        nc.scalar.copy(
            e2.rearrange("d a c -> d (a c)"), e.rearrange("d a c -> d (a c)")
        )
    else:
        rinv = bufs["rinv_static"]

    if "noapp" in ablate:
        return

    # ---- conv3 into padded buffer (PE busy while softmax runs).  Computed
    # in grid order like conv1/2 (contiguous rhs); the psum->sbuf copy
    # scatters each grid back into the natural y3_pad layout.  The E
    # transposes are emitted after the first 2 conv3 k's: late enough that
    # e2 is ready (softmax hides under ~8us of conv3 PE work), early enough
    # that their ACT et-copies don't queue behind all 18 psum scatters.
    def conv3_k(k):
        kh, kw = divmod(k, 3)
        for g in range(n_gt):
            h0 = g * gh
            pc = psma.tile([128, gn], F32, tag="mma", name="pc3")
            for d in range(9):
                nc.tensor.matmul(
                    pc, w_sb[2][:, d, :], grid_rhs(k, g, d),
                    start=(d == 0), stop=(d == 8),
                )
            r0, c0 = 1 + kh + 3 * h0, 1 + kw
            nc.scalar.copy(
                y3_pad[
                    :,
                    r0 : r0 + 3 * (gh - 1) + 1 : 3,
                    c0 : c0 + 3 * (wb - 1) + 1 : 3,
                ],
                pc[:C].rearrange("c (a b) -> c a b", a=gh),
            )

    for k in range(2):
        conv3_k(k)

    # ---- transpose the 9 [d, c] softmax slices to [c, d]
    if "noc12" not in ablate:
        et = bufs["et_b"]
        for k in range(9):
            pe = pstre.tile([128, C], mm_dt, tag="tr", name="pe")
            nc.tensor.transpose(pe[:C, :], e2[:, k, :], ident)
            nc.scalar.copy(et[:, k, :C], pe[:C, :])
    else:
        et = bufs["et_static"]

    for k in range(2, 9):
        conv3_k(k)

    # ---- apply: out[d, hw] = rinv[d] * sum_{k,c} et[c,k,d] * y3p[c, hw+k]
    for h0, r in cfg["tiles_rt"]:
        pa = psma.tile([128, r * w], F32, tag="mma", name="pa")
        for k in range(9):
            kh, kw = divmod(k, 3)
            rhs = y3_pad[:, h0 + kh : h0 + kh + r, kw : kw + w]
            nc.tensor.matmul(pa, et[:, k, :], rhs, start=(k == 0), stop=(k == 8))
        ob = work.tile([C, r * w], F32, tag="ob", name="ob")
        nc.scalar.activation(
            ob, pa[:C], mybir.ActivationFunctionType.Copy, scale=rinv
        )
        nc.sync.dma_start(out=out_f[s][:, h0 * w : (h0 + r) * w], in_=ob)


_CACHE = {}


def _get_program():
    if "nc" not in _CACHE:
        _CACHE["nc"] = build_program()
    return _CACHE["nc"]


def prep_arrays(x, W1, W2, W3, mm_dt=MM_DT):
    """Full (unsharded) input arrays keyed by DRAM tensor name.  x is sent in
    zero-padded grid layout [b, c, 9, hb+2, wb+2] with grid k=(kh,kw) plane
    holding pixels (3*hb+kh, 3*wb+kw), split into two contiguous row bands
    (A: rows [0, gh+2), B: rows [gh, hb+2)) for fast banded DMA."""
    np_in = np.float32 if mm_dt == F32R else np.float16
    x = np.asarray(x, dtype=np.float32)
    b, c, h, w = x.shape
    hb, wb = h // 3, w // 3
    gh = min(hb, 512 // wb)
    xg = np.zeros((b, c, 9, hb + 2, wb + 2), dtype=np_in)
    xg[:, :, :, 1:-1, 1:-1] = (
        x.reshape(b, c, hb, 3, wb, 3)
        .transpose(0, 1, 3, 5, 2, 4)
        .reshape(b, c, 9, hb, wb)
    )
    arrs = {}
    for i, Wi in ((1, W1), (2, W2), (3, W3)):
        wt = np.zeros((c, 9, 128), dtype=np_in)
        wt[:, :, :c] = (
            np.asarray(Wi, dtype=np.float32).transpose(1, 2, 3, 0).reshape(c, 9, c)
        ).astype(np_in)
        arrs[f"w{i}t"] = wt
    if hb // gh == 2 and mm_dt != F32R:
        arrs["xpa"] = np.ascontiguousarray(xg[:, :, :, : gh + 2])
        arrs["xpb"] = np.ascontiguousarray(xg[:, :, :, gh:])
    else:
        arrs["xp"] = xg
    arrs["ident"] = np.eye(c, dtype=np_in)
    return arrs


def prep_in_maps(x, W1, W2, W3, mm_dt=MM_DT):
    arrs = prep_arrays(x, W1, W2, W3, mm_dt)
    b = np.asarray(x).shape[0]
    s = b // NCORES
    return [
        {
            k: (v[cr * s : (cr + 1) * s] if k in ("xp", "xpa", "xpb") else v)
            for k, v in arrs.items()
        }
        for cr in range(NCORES)
    ]


def run(x, W1, W2, W3, **spmd_kwargs):
    from concourse.bass_utils import run_bass_kernel_spmd

    in_maps = prep_in_maps(x, W1, W2, W3)
    nc = _get_program()
    res = run_bass_kernel_spmd(nc, in_maps, core_ids=list(range(NCORES)), **spmd_kwargs)
    out = np.concatenate([res.results[c]["out"] for c in range(NCORES)], axis=0)
    return out.astype(np.float32), res


def kernel(x, W1, W2, W3):
    return run(x, W1, W2, W3)[0]



# revision 22
# speedup vs baseline: 1.0186x; 1.0186x over previous
"""Trainium2 Bass kernel for nn_AttnConv2d (sparse_attention).

Reference math (per sample b):
  y1 = conv3x3(x, W1), y2 = conv3x3(x, W2), y3 = conv3x3(x, W3)   (same pad)
  For each of the 9 offsets k=(kh,kw): the "grid_k" pixels are
  (3*hb+kh, 3*wb+kw).  z[d, k, c] = sum_{l in grid_k} y1[c, l] * y2[d, l].
  P = softmax over (k,c) of z / sqrt(864), per row d.
  out[d, h, w] = sum_{c, (i,j)} P[d, i*3+j, c] * y3[c, h+i-1, w+j-1]
  (i.e. a 3x3 conv of y3 with per-sample weights from the softmax).

Implementation notes:
 - Data parallel over batch: 16 samples -> 8 cores x 2 samples.
 - Matmul dtype is configurable: float16 (default; 1 PE cycle/row,
   ~1e-3 rel err) or float32r (~4e-4 rel err but the embedded weight
   load of fp32-class matmuls serializes, costing ~30%).
 - fp32r operands must be produced by a rounding compute op (DVE copy),
   never directly by DMA; fp16 has no such constraint so inputs DMA in
   pre-converted from the host.
 - conv1/conv2 are computed in grid-k pixel order; their [l, c]
   transposed chunks (gram operands need pixels on partitions) are made
   by the DMA XBAR (dma_start_transpose, fp16 SBUF->SBUF, one batched
   trigger per conv tile) so they cost no PE cycles.  A deep (14) st
   staging ring absorbs the slow-transpose window while the x-prefetch
   shares the DMA engines.
 - PE program order per sample: conv1+conv2 -> gram -> conv3
   -> E transposes -> apply.  conv3 sits between gram and E-transposes
   so the softmax (ACT/DVE) latency hides under PE work.
 - ~5us of dummy warmup matmuls at kernel start keep the PE busy through
   the input-DMA lead-in so the HAM clock gate is at 2.4 GHz (not the
   cold 1.2) when the first real conv matmul issues.
 - Sample-0 band A is split across both hwdge queues (scalar + sync
   trigger engines) so it lands ~1.2us sooner; the first real matmul
   fires ~14us in, bounded by NEFF preamble (~7us) + DMA ring startup
   (~2us) + the 1.2MB critical transfer at ~250 GB/s.
 - Stationary weights are host-padded to 128 couts (zeros beyond 96) to
   hit the compiler's fast-weight-load path; psum tiles are [128, n] with
   consumers reading [:96].
 - PSUM banks: conv 2 + conv3/apply 3 + E-transpose 1 + gram 2 (1 buf
   serialized the gram on its z copy).
 - Measured: PE issue stream is gapless (<2us total idle over 312us);
   conv matmuls run at ~228ns/512cols (~6% over the 213.5 streaming
   floor, from per-row-jump cost of the strided x-window reads — padding
   the stream to whole rows costs more than the jumps save).
"""

import contextlib
import math

import numpy as np

import concourse.bacc as bacc
import concourse.bass as bass
import concourse.mybir as mybir
import concourse.tile as tile

F32 = mybir.dt.float32
F32R = mybir.dt.float32r
FP16 = mybir.dt.float16

B, C, H, W = 16, 96, 96, 96
NCORES = 8
S = B // NCORES  # samples per core
MM_DT = FP16  # conv / apply matmuls (fp16 ~ fp32r speed, no rounding funnel)
TR_DT = FP16  # conv1/2-output transposes + gram matmuls


def build_program(
    s_per_core=S, h=H, w=W, mm_dt=MM_DT, reps=1, hw_loop=None, ablate=""
):
    hp, wp = h + 2, w + 2
    hb, wb = h // 3, w // 3  # grid blocks per offset k
    gp, gw = hb + 2, wb + 2  # zero-padded grid block planes
    padpix = 9 * gp * gw  # x arrives in grid layout [9, gp, gw]
    L = hb * wb  # pixels per grid
    nch = (L + 127) // 128  # 128-pixel transpose chunks per grid
    # conv tiling in grid order: gh block-rows per PSUM tile (<=512 px)
    gh = min(hb, 512 // wb)
    n_gt = (hb + gh - 1) // gh
    gn = gh * wb
    assert gn % 128 == 0 or n_gt == 1, (gn, n_gt)
    # apply natural-order tiling: rt rows per tile (<=512 px), ragged tail
    rt = max(1, min(h, 512 // w))
    tiles_rt = [(h0, min(rt, h - h0)) for h0 in range(0, h, rt)]
    n_rt = len(tiles_rt)
    sc = 1.0 / math.sqrt(C * 9)
    rounded = mm_dt == F32R  # fp32r needs DVE-rounded producers
    in_dt = F32 if rounded else mm_dt

    split_x = "nox" not in ablate and not rounded and hb // gh == 2
    nc = bacc.Bacc("TRN2", debug=False, enable_asserts=False)
    if split_x:
        # x arrives as two contiguous row bands (A: rows [0,gh+2) for g=0,
        # B: rows [gh,gp) for g=1) so the input DMAs use big descriptors.
        xpa_d = nc.dram_tensor(
            "xpa", [s_per_core, C, 9, gh + 2, gw], in_dt, kind="ExternalInput"
        )
        xpb_d = nc.dram_tensor(
            "xpb", [s_per_core, C, 9, gp - gh, gw], in_dt, kind="ExternalInput"
        )
        xp_d = None
    else:
        xp_d = nc.dram_tensor(
            "xp", [s_per_core, C, 9, gp, gw], in_dt, kind="ExternalInput"
        )
    # weights host-padded to 128 couts (zeros beyond C): a 128-column
    # stationary triggers the compiler's Fast Weight Load path, halving
    # LDWEIGHTS time so it fully hides under the matmul stream.
    MP = 128
    w_d = [
        nc.dram_tensor(f"w{i}t", [C, 9, MP], in_dt, kind="ExternalInput")
        for i in (1, 2, 3)
    ]
    id_d = nc.dram_tensor("ident", [C, C], in_dt, kind="ExternalInput")
    out_d = nc.dram_tensor("out", [s_per_core, C, h, w], F32, kind="ExternalOutput")

    xp_f = None if split_x else xp_d.ap().rearrange("s c a h w -> s c (a h w)")
    out_f = out_d.ap().rearrange("s c h w -> s c (h w)")

    xchunk = (padpix + 3) // 4
    cfg = dict(
        h=h, w=w, hp=hp, wp=wp, gp=gp, gw=gw, padpix=padpix, xchunk=xchunk,
        hb=hb, wb=wb, L=L, nch=nch, gh=gh, n_gt=n_gt, gn=gn, rt=rt, n_rt=n_rt,
        tiles_rt=tiles_rt,
        sc=sc, mm_dt=mm_dt, rounded=rounded, ablate=ablate, split_x=split_x,
    )

    with tile.TileContext(nc) as tc:
        with (
            tc.tile_pool(name="consts", bufs=1) as consts,
            tc.tile_pool(name="stage", bufs=2) as stage,
            tc.tile_pool(name="big", bufs=1) as big,
            tc.tile_pool(name="work", bufs=3) as work,
            tc.tile_pool(name="sm", bufs=1) as smp,
            tc.tile_pool(name="psmm", bufs=2, space="PSUM") as psmm,
            tc.tile_pool(name="psma", bufs=3, space="PSUM") as psma,
            tc.tile_pool(name="pstr", bufs=1, space="PSUM") as pstr,
            tc.tile_pool(name="psgr", bufs=2, space="PSUM") as psgr,
        ):
            # E transposes share the pstr ring (same tag) — frees a PSUM
            # bank so the gram pool gets 2 bufs (its copy was serializing).
            pools = dict(
                consts=consts, stage=stage, big=big, work=work, smp=smp,
                psmm=psmm, psma=psma, pstr=pstr, pstre=pstr, psgr=psgr,
            )
            if "nowarm" not in ablate:
                # ~5us of dummy matmuls at kernel start: keeps PE busy
                # through the input-DMA lead-in so the HAM clock gate is
                # warm (2.4 GHz) before the first real conv matmul.
                warm = consts.tile([128, 96], mm_dt, tag="warm", name="warm")
                nc.vector.memset(warm, 0.0)
                wps = psgr.tile([96, 96], F32, tag="gram", name="wps")
                for _ in range(120):
                    nc.tensor.matmul(wps, warm[:, :96], warm, start=True, stop=True)
            # ---- constants: weights (host-transposed [cin, d, cout]), identity
            x0 = None
            w_sb = []
            for i, wd in enumerate(w_d):
                if rounded:
                    ws = stage.tile([C, 9 * MP], F32, tag="wstage", name=f"ws{i}")
                    nc.sync.dma_start(
                        out=ws, in_=wd.ap().rearrange("c a d -> c (a d)")
                    )
                    wt = consts.tile([C, 9, MP], mm_dt, tag=f"w{i}", name=f"w{i}")
                    nc.vector.tensor_copy(wt.rearrange("c a d -> c (a d)"), ws)
                else:
                    wt = consts.tile([C, 9, MP], mm_dt, tag=f"w{i}", name=f"w{i}")
                    nc.sync.dma_start(out=wt, in_=wd.ap())
                w_sb.append(wt)
                if i == 0 and split_x:
                    # sample-0 band A split across BOTH hwdge queues (each
                    # trigger engine owns one queue): scalar's queue takes 5
                    # planes while sync's takes w1 then 4 planes — band A
                    # lands ~2.5us sooner than single-queue serial.
                    xa0 = big.tile(
                        [C, 9, gh + 2, gw], mm_dt, tag="x_pad_a", bufs=2,
                        name="xa",
                    )
                    xb0 = big.tile(
                        [C, 9, gp - gh, gw], mm_dt, tag="x_pad_b", bufs=2,
                        name="xb",
                    )
                    nc.scalar.dma_start(out=xa0[:, 0:5], in_=xpa_d.ap()[0][:, 0:5])
                    nc.sync.dma_start(out=xa0[:, 5:9], in_=xpa_d.ap()[0][:, 5:9])
                    x0 = (xa0, xb0)
                if i == 1 and split_x:
                    # xb after w2 on sync: deadline is the first g=1 tile
                    # (~35us in); w3/ident can trail it.
                    nc.sync.dma_start(out=x0[1], in_=xpb_d.ap()[0])
            ids = stage.tile([C, C], F32, tag="wstage", name="ids")
            if rounded:
                nc.sync.dma_start(out=ids, in_=id_d.ap())
                ident = consts.tile([C, C], mm_dt, tag="ident")
                nc.vector.tensor_copy(ident, ids)
            else:
                ident = consts.tile([C, C], mm_dt, tag="ident")
                # scalar queue: lands before sync finishes w2/w3, so the
                # first conv transposes (~4us after first matmul) aren't
                # gated on the weight-DMA tail.
                nc.scalar.dma_start(out=ident, in_=id_d.ap())
                nc.vector.tensor_copy(ids, ident)
            ident_h = consts.tile([C, C], TR_DT, tag="ident_h")
            nc.vector.tensor_copy(ident_h, ids)
            zerof = consts.tile([C, max(hp, wp)], F32, tag="zerof")
            nc.vector.memset(zerof, 0.0)

            # ---- persistent per-core buffers
            y3_pad = big.tile([C, hp, wp], mm_dt, tag="y3_pad")
            # apply stationary [C, 9, 128]: cols >= C stay zero so the
            # 128-col weight load (FWL) is valid; ACT writes [:, k, :C].
            et_b = big.tile([C, 9, 128], mm_dt, tag="et_b")
            nc.vector.memset(et_b.rearrange("c a d -> c (a d)"), 0.0)
            y1t = big.tile([128, 9, nch, C], TR_DT, tag="y1t")
            y2t = big.tile([128, 9, nch, C], TR_DT, tag="y2t")
            bufs = dict(
                y3_pad=y3_pad, y1t=y1t, y2t=y2t,
                y1t_f=y1t.rearrange("p a b c -> p (a b c)"), et_b=et_b,
                w_sb=w_sb, ident=ident, ident_h=ident_h,
                xp_f=xp_f, out_f=out_f, x0=x0,
                xpa=xpa_d.ap() if split_x else None,
                xpb=xpb_d.ap() if split_x else None,
            )

            # y3_pad borders are zero once (never overwritten: conv3 writes
            # the interior only, and the single buffer persists).
            nc.vector.tensor_copy(y3_pad[:, 0, :], zerof[:, :wp])
            nc.vector.tensor_copy(y3_pad[:, hp - 1, :], zerof[:, :wp])
            nc.vector.tensor_copy(y3_pad[:, :, 0], zerof[:, :hp])
            nc.vector.tensor_copy(y3_pad[:, :, wp - 1], zerof[:, :hp])

            if "nox" in ablate:
                # load x once, outside the timing loop
                x_pad1 = big.tile([C, 9, gp, gw], mm_dt, tag="x_pad", name="x_pad1")
                xf = x_pad1.rearrange("c a h w -> c (a h w)")
                for o in range(0, padpix, xchunk):
                    cl = min(xchunk, padpix - o)
                    xs = stage.tile([C, xchunk], F32, tag="xs", name="xs")
                    nc.sync.dma_start(out=xs[:, :cl], in_=xp_f[0][:, o : o + cl])
                    nc.vector.tensor_copy(xf[:, o : o + cl], xs[:, :cl])
                bufs["x_pad_static"] = x_pad1
            if "noc12" in ablate:
                et_s = big.tile([C, 9, C], mm_dt, tag="et_s", name="et_s")
                for k in range(9):
                    nc.vector.tensor_copy(et_s[:, k, :], zerof[:, :C])
                rinv_s = big.tile([C, 1], F32, tag="rinv_s", name="rinv_s")
                nc.vector.memset(rinv_s, 1.0)
                bufs["et_static"], bufs["rinv_static"] = et_s, rinv_s

            loop_ctx = (
                tc.For_i(0, hw_loop, 1)
                if hw_loop is not None
                else contextlib.nullcontext()
            )
            with loop_ctx:
                for s in [s for _ in range(reps) for s in range(s_per_core)]:
                    _emit_sample(nc, s, pools, bufs, cfg)
    nc.finalize()
    return nc


def _emit_sample(nc, s, pools, bufs, cfg):
    stage, work, smp, big = pools["stage"], pools["work"], pools["smp"], pools["big"]
    psmm, psma, pstr, pstre, psgr = (
        pools["psmm"], pools["psma"], pools["pstr"], pools["pstre"], pools["psgr"]
    )
    y3_pad, y1t, y2t = bufs["y3_pad"], bufs["y1t"], bufs["y2t"]
    y1t_f, w_sb, ident, ident_h = (
        bufs["y1t_f"], bufs["w_sb"], bufs["ident"], bufs["ident_h"]
    )
    xp_f, out_f = bufs["xp_f"], bufs["out_f"]
    hp, wp, gp, gw, padpix, xchunk = (
        cfg["hp"], cfg["wp"], cfg["gp"], cfg["gw"], cfg["padpix"], cfg["xchunk"]
    )
    w = cfg["w"]
    wb, L, nch, gh, n_gt, gn, rt, n_rt, sc = (
        cfg["wb"], cfg["L"], cfg["nch"], cfg["gh"], cfg["n_gt"], cfg["gn"],
        cfg["rt"], cfg["n_rt"], cfg["sc"],
    )
    mm_dt, rounded, ablate = cfg["mm_dt"], cfg["rounded"], cfg["ablate"]

    # ---- load x (host sends the zero-padded GRID layout [9, gp, gw]; fp32r
    # needs a DVE rounding hop).  Double-buffered so the next sample's load
    # overlaps compute.  The load is split into two row-band tiles (g=0
    # rows / g=1 rows, 1-row overlap) on separate DMAs, so with the
    # g-outer conv loop below PE starts after the first band lands.
    split_x = cfg["split_x"]
    if "nox" in ablate:
        x_pad = bufs["x_pad_static"]
    elif split_x and s == 0 and bufs.get("x0") is not None:
        xa, xb = bufs["x0"]  # preloaded at program start
    elif split_x:
        ra = gh + 2  # band A rows [0, gh+2): covers g=0 reads (q in -1..1)
        rb = gp - gh  # band B rows [gh, gp): covers g=1 reads
        xa = big.tile([C, 9, ra, gw], mm_dt, tag="x_pad_a", bufs=2, name="xa")
        xb = big.tile([C, 9, rb, gw], mm_dt, tag="x_pad_b", bufs=2, name="xb")
        # band A feeds the first PE work: 3 contiguous-plane DMAs on separate
        # queues so it lands ~3x sooner; band B has a late deadline (g=1).
        for a0 in range(0, 9, 3):
            nc.sync.dma_start(
                out=xa[:, a0 : a0 + 3], in_=bufs["xpa"][s][:, a0 : a0 + 3]
            )
        nc.sync.dma_start(out=xb, in_=bufs["xpb"][s])
    else:
        x_pad = big.tile([C, 9, gp, gw], mm_dt, tag="x_pad", bufs=2, name="x_pad")
        x_pad_f = x_pad.rearrange("c a h w -> c (a h w)")
        if rounded:
            for o in range(0, padpix, xchunk):
                cl = min(xchunk, padpix - o)
                xs = stage.tile([C, xchunk], F32, tag="xs", name="xs")
                nc.sync.dma_start(out=xs[:, :cl], in_=xp_f[s][:, o : o + cl])
                nc.vector.tensor_copy(x_pad_f[:, o : o + cl], xs[:, :cl])
        else:
            nc.sync.dma_start(out=x_pad_f, in_=xp_f[s])

    # Contiguous (pad-stripped) copies of the x bands: 63/81 tap-streams
    # have zero column shift (qc==0) and can stream [gh, wb] windows with
    # NO row jumps from these, dropping the ~13ns/MM strided-read tax on
    # 7/9 of the conv matmuls.  The qc=+-1 edge taps keep reading the
    # padded bands.  Built by two DVE copies (~2.2us each) right after
    # the band DMAs; sample 0's first conv tiles read the padded path
    # while the copy drains.
    use_xc = split_x and "xc" in ablate
    if use_xc:
        ra_, rb_ = gh + 2, gp - gh
        xa_c = big.tile([C, 9, ra_, wb], mm_dt, tag="xa_c", bufs=2, name="xa_c")
        nc.vector.tensor_copy(xa_c, xa[:, :, :, 1 : 1 + wb])
        xb_c = big.tile([C, 9, rb_, wb], mm_dt, tag="xb_c", bufs=2, name="xb_c")
        # ACT, not DVE: DVE is in-order and this waits on band B's DMA
        # (~20us) — on DVE it would block the conv CASTs behind it.
        nc.scalar.copy(xb_c, xb[:, :, :, 1 : 1 + wb])

    def grid_rhs(k, g, d, allow_c=True):
        """x view for output-grid k, block-row tile g, conv offset d."""
        kh, kw = divmod(k, 3)
        dh, dw = divmod(d, 3)
        q, r = divmod(kh + dh - 1, 3)
        qc, rc = divmod(kw + dw - 1, 3)
        h0 = g * cfg["gh"]
        if split_x:
            r0 = 1 + h0 + q - (0 if g == 0 else gh)
            if qc == 0 and allow_c and use_xc:
                srcc = xa_c if g == 0 else xb_c
                return srcc[:, r * 3 + rc, r0 : r0 + gh, :]
            src = xa if g == 0 else xb
            return src[:, r * 3 + rc, r0 : r0 + gh, 1 + qc : 1 + qc + wb]
        return x_pad[
            :, r * 3 + rc, 1 + h0 + q : 1 + h0 + q + gh, 1 + qc : 1 + qc + wb
        ]

    # ---- conv1 + conv2 in grid-k order, transposed into y1t/y2t.  The
    # transposes go through the DMA XBAR (dma_start_transpose, fp16
    # SBUF->SBUF) so they cost no PE cycles; triggers alternate between
    # the two hwdge queues (sync/scalar), which are otherwise idle in the
    # conv phase.  "nodmatr" ablate falls back to PE transposes + DVE
    # copies (one conv-tile behind so PE never waits on the CAST).
    n_ch_g = (gn + 127) // 128
    use_dmatr = (
        "nodmatr" not in ablate and gn % 128 == 0 and C % 16 == 0
    )

    def emit_transposes(pending):
        st, k_, yt_, g_ = pending
        ci0 = g_ * n_ch_g
        if use_dmatr:
            eng = nc.sync if k_ % 2 == 0 else nc.scalar
            if "dmatr1" in ablate:  # per-chunk 2D fallback
                for j in range(n_ch_g):
                    eng.dma_start_transpose(
                        out=yt_[:, k_, ci0 + j, :],
                        in_=st[:, j * 128 : (j + 1) * 128],
                    )
            else:  # batched: one trigger per tile, 3D out AP
                eng.dma_start_transpose(
                    out=yt_[:, k_, ci0 : ci0 + n_ch_g, :], in_=st
                )
            return
        pt = pstr.tile([128, n_ch_g, C], TR_DT, tag="tr", name="pt")
        for j, j0 in enumerate(range(0, gn, 128)):
            cs = min(128, gn - j0)
            nc.tensor.transpose(pt[:cs, j, :], st[:, j0 : j0 + cs], ident_h)
        cs0 = min(128, gn - (n_ch_g - 1) * 128)  # partitions of last chunk
        if cs0 == 128:
            nc.vector.tensor_copy(
                yt_[:, k_, ci0 : ci0 + n_ch_g, :].rearrange("p a c -> p (a c)"),
                pt.rearrange("p a c -> p (a c)"),
            )
        else:  # small-config fallback: per-chunk copies of valid partitions
            for j in range(n_ch_g):
                cs = min(128, gn - j * 128)
                nc.vector.tensor_copy(yt_[:cs, k_, ci0 + j, :], pt[:cs, j, :])

    # g-outer so all g=0 tiles run off band A while band B still streams in.
    # Sample 0's first tiles keep the padded-read path so the PE isn't
    # gated on the pad-strip DVE copy (~2us after band A lands).
    pending = None
    tile_no = 0
    for g in range(n_gt) if "noc12" not in ablate else ():
        for k in range(9):
            for wt, yt in ((w_sb[0], y1t), (w_sb[1], y2t)):
                allow_c = s > 0 or g == 1
                tile_no += 1
                pc = psmm.tile([128, gn], F32, tag="mm", name="pc")
                for d in range(9):
                    nc.tensor.matmul(
                        pc, wt[:, d, :], grid_rhs(k, g, d, allow_c),
                        start=(d == 0), stop=(d == 8),
                    )
                # 10-deep staging ring: during the first ~30us the XBAR
                # transpose transfers share DMA engines with the 4MB
                # x-prefetch and run slow; a deep ring keeps the CASTs (and
                # therefore the psum banks and conv matmuls) from blocking
                # on them.  Deadline is only the gram (~75us in).
                st = work.tile([C, gn], TR_DT, tag="st", name="st", bufs=14)
                nc.vector.tensor_copy(st, pc[:C])
                if pending is not None:
                    emit_transposes(pending)
                pending = (st, k, yt, g)
    if pending is not None:
        emit_transposes(pending)

    # ---- gram: z[d, k, c] = sum_l y2t[l,d] * y1t[l,c]
    z = smp.tile([C, 9, C], F32, tag="z", name="z")
    gram_n = C  # fp16 gram runs full-rate at any moving size
    for k in range(9) if "noc12" not in ablate else ():
        pg = psgr.tile([C, gram_n], F32, tag="gram", name="pg")
        for ci in range(nch):
            cs = min(128, L - ci * 128)
            off = (k * nch + ci) * C
            n = min(gram_n, 9 * nch * C - off)
            nc.tensor.matmul(
                pg[:, :n],
                y2t[:cs, k, ci, :],
                y1t_f[:cs, off : off + n],
                start=(ci == 0),
                stop=(ci == nch - 1),
            )
        nc.vector.tensor_copy(z[:, k, :], pg[:, :C])

    # ---- softmax over (k, c) per row d; normalization deferred to the
    # output scale (rinv folded into the final psum->sbuf copy).
    if "noc12" not in ablate:
        zf = z.rearrange("d a c -> d (a c)")
        mneg = smp.tile([C, 1], F32, tag="mneg", name="mneg")
        nc.vector.reduce_max(
            out=mneg, in_=zf, axis=mybir.AxisListType.X, negate=True
        )
        mns = smp.tile([C, 1], F32, tag="mns", name="mns")
        nc.vector.tensor_scalar_mul(mns, mneg, sc)
        e = smp.tile([C, 9, C], F32, tag="e", name="e")
        r = smp.tile([C, 1], F32, tag="r", name="r")
        nc.scalar.activation(
            e.rearrange("d a c -> d (a c)"),
            zf,
            mybir.ActivationFunctionType.Exp,
            bias=mns,
            scale=sc,
            accum_out=r,
        )
        rinv = smp.tile([C, 1], F32, tag="rinv", name="rinv")
        nc.vector.reciprocal(rinv, r)
        # e2: ACT-produced rounding copy; the whole conv3/apply side (y3, et,
        # e2, output scaling) funnels through ACT so each matmul there waits
        # on a single semaphore, while conv1/2+gram funnel through DVE.
        e2 = smp.tile([C, 9, C], mm_dt, tag="e2", name="e2")
        nc.scalar.copy(
            e2.rearrange("d a c -> d (a c)"), e.rearrange("d a c -> d (a c)")
        )
    else:
        rinv = bufs["rinv_static"]

    if "noapp" in ablate:
        return

    # ---- conv3 into padded buffer (PE busy while softmax runs).  Computed
    # in grid order like conv1/2 (contiguous rhs); the psum->sbuf copy
    # scatters each grid back into the natural y3_pad layout.  The E
    # transposes are emitted after the first 2 conv3 k's: late enough that
    # e2 is ready (softmax hides under ~8us of conv3 PE work), early enough
    # that their ACT et-copies don't queue behind all 18 psum scatters.
    def conv3_k(k):
        kh, kw = divmod(k, 3)
        for g in range(n_gt):
            h0 = g * gh
            pc = psma.tile([128, gn], F32, tag="mma", name="pc3")
            for d in range(9):
                nc.tensor.matmul(
                    pc, w_sb[2][:, d, :], grid_rhs(k, g, d),
                    start=(d == 0), stop=(d == 8),
                )
            r0, c0 = 1 + kh + 3 * h0, 1 + kw
            nc.scalar.copy(
                y3_pad[
                    :,
                    r0 : r0 + 3 * (gh - 1) + 1 : 3,
                    c0 : c0 + 3 * (wb - 1) + 1 : 3,
                ],
                pc[:C].rearrange("c (a b) -> c a b", a=gh),
            )

    for k in range(2):
        conv3_k(k)

    # ---- transpose the 9 [d, c] softmax slices to [c, d]
    if "noc12" not in ablate:
        et = bufs["et_b"]
        for k in range(9):
            pe = pstre.tile([128, C], mm_dt, tag="tr", name="pe")
            nc.tensor.transpose(pe[:C, :], e2[:, k, :], ident)
            nc.scalar.copy(et[:, k, :C], pe[:C, :])
    else:
        et = bufs["et_static"]

    for k in range(2, 9):
        conv3_k(k)

    # ---- apply: out[d, hw] = rinv[d] * sum_{k,c} et[c,k,d] * y3p[c, hw+k]
    for h0, r in cfg["tiles_rt"]:
        pa = psma.tile([128, r * w], F32, tag="mma", name="pa")
        for k in range(9):
            kh, kw = divmod(k, 3)
            rhs = y3_pad[:, h0 + kh : h0 + kh + r, kw : kw + w]
            nc.tensor.matmul(pa, et[:, k, :], rhs, start=(k == 0), stop=(k == 8))
        ob = work.tile([C, r * w], F32, tag="ob", name="ob")
        nc.scalar.activation(
            ob, pa[:C], mybir.ActivationFunctionType.Copy, scale=rinv
        )
        nc.sync.dma_start(out=out_f[s][:, h0 * w : (h0 + r) * w], in_=ob)


_CACHE = {}


def _get_program():
    if "nc" not in _CACHE:
        _CACHE["nc"] = build_program()
    return _CACHE["nc"]


def prep_arrays(x, W1, W2, W3, mm_dt=MM_DT):
    """Full (unsharded) input arrays keyed by DRAM tensor name.  x is sent in
    zero-padded grid layout [b, c, 9, hb+2, wb+2] with grid k=(kh,kw) plane
    holding pixels (3*hb+kh, 3*wb+kw), split into two contiguous row bands
    (A: rows [0, gh+2), B: rows [gh, hb+2)) for fast banded DMA."""
    np_in = np.float32 if mm_dt == F32R else np.float16
    x = np.asarray(x, dtype=np.float32)
    b, c, h, w = x.shape
    hb, wb = h // 3, w // 3
    gh = min(hb, 512 // wb)
    xg = np.zeros((b, c, 9, hb + 2, wb + 2), dtype=np_in)
    xg[:, :, :, 1:-1, 1:-1] = (
        x.reshape(b, c, hb, 3, wb, 3)
        .transpose(0, 1, 3, 5, 2, 4)
        .reshape(b, c, 9, hb, wb)
    )
    arrs = {}
    for i, Wi in ((1, W1), (2, W2), (3, W3)):
        wt = np.zeros((c, 9, 128), dtype=np_in)
        wt[:, :, :c] = (
            np.asarray(Wi, dtype=np.float32).transpose(1, 2, 3, 0).reshape(c, 9, c)
        ).astype(np_in)
        arrs[f"w{i}t"] = wt
    if hb // gh == 2 and mm_dt != F32R:
        arrs["xpa"] = np.ascontiguousarray(xg[:, :, :, : gh + 2])
        arrs["xpb"] = np.ascontiguousarray(xg[:, :, :, gh:])
    else:
        arrs["xp"] = xg
    arrs["ident"] = np.eye(c, dtype=np_in)
    return arrs


def prep_in_maps(x, W1, W2, W3, mm_dt=MM_DT):
    arrs = prep_arrays(x, W1, W2, W3, mm_dt)
    b = np.asarray(x).shape[0]
    s = b // NCORES
    return [
        {
            k: (v[cr * s : (cr + 1) * s] if k in ("xp", "xpa", "xpb") else v)
            for k, v in arrs.items()
        }
        for cr in range(NCORES)
    ]


def run(x, W1, W2, W3, **spmd_kwargs):
    from concourse.bass_utils import run_bass_kernel_spmd

    in_maps = prep_in_maps(x, W1, W2, W3)
    nc = _get_program()
    res = run_bass_kernel_spmd(nc, in_maps, core_ids=list(range(NCORES)), **spmd_kwargs)
    out = np.concatenate([res.results[c]["out"] for c in range(NCORES)], axis=0)
    return out.astype(np.float32), res


def kernel(x, W1, W2, W3):
    return run(x, W1, W2, W3)[0]

